# revision 6
# baseline (speedup 1.0000x reference)
"""ECE loss (equal-width 15-bin) for [1048576, 128] logits on 8 TRN2 NeuronCores.

Strategy (data-parallel over rows, per the sharding hint):
  Device, per core (N/8 = 131072 rows):
    - stream [128 partitions, G rows, 128 classes] supertiles of y_pred
    - DVE:   grouped reduce_max over classes -> per-row max m (all rows)
    - row softmax denominators U = sum_c exp(x_c) (unshifted exp is safe:
      |x| <= ~6.5), split between two engines to balance their load:
        * rows [0, KA) of each supertile: one ACT activation per row with
          accum_out -> exp+sum fused on the Scalar engine
        * rows [KA, G): one batched ACT exp + one grouped DVE reduce_sum
    - outputs m, u_a, u_b -- a 512MB -> 1.5MB reduction
  Host:
    conf = exp(m)/U  (== max softmax);  acc = (y_pred[r, y_true[r]] == m)
    (the row max is an exact element of the row, so float equality
    reproduces argmax == label up to exact-tie rows), then the 15-bin
    equal-width histogram and the final ECE reduction as in the reference.

Measured v1 (all reductions on DVE): 311us; DVE busy 282us was the
bottleneck while ACT sat at 119us and the DMA floor is ~195us. The KA
split moves ~half the row-sums to ACT's idle capacity.
"""

import numpy as np

import concourse.bacc as bacc
import concourse.tile as tile
from concourse import mybir
from concourse.bass_utils import run_bass_kernel_spmd

N_CORES = 8
N = 1048576
C = 128
N_SHARD = N // N_CORES  # 131072
P = 128                 # SBUF partitions
T = N_SHARD // P        # 1024 rows handled per partition
G = 32                  # rows per partition per supertile
N_ST = T // G           # supertiles per core
KA = 16                 # rows per supertile whose U is computed on ACT (accum)
KB = G - KA             # rows per supertile whose U is computed on DVE
N_BINS = 15

_CACHE: dict = {}


def _build_bass():
    nc = bacc.Bacc(None, target_bir_lowering=False)
    x = nc.dram_tensor("x", [N_SHARD, C], mybir.dt.float32, kind="ExternalInput")
    m_out = nc.dram_tensor("m_out", [N_SHARD], mybir.dt.float32, kind="ExternalOutput")
    ua_out = nc.dram_tensor("ua_out", [P * N_ST * KA], mybir.dt.float32, kind="ExternalOutput")
    ub_out = nc.dram_tensor("ub_out", [P * N_ST * KB], mybir.dt.float32, kind="ExternalOutput")

    # row r = p*T + t lives at [p, t]; per-partition runs in DRAM stay contiguous
    xv = x[:, :].rearrange("(p t) c -> p t c", p=P)
    mv = m_out[:].rearrange("(p t) -> p t", p=P)
    uav = ua_out[:].rearrange("(p t) -> p t", p=P)
    ubv = ub_out[:].rearrange("(p t) -> p t", p=P)

    with tile.TileContext(nc) as tc:
        with (
            tc.tile_pool(name="xin", bufs=5) as xin_pool,
            tc.tile_pool(name="exps", bufs=3) as exp_pool,
            tc.tile_pool(name="stats", bufs=1) as stats_pool,
        ):
            m_all = stats_pool.tile([P, T], mybir.dt.float32)
            ua_all = stats_pool.tile([P, N_ST * KA], mybir.dt.float32)
            ub_all = stats_pool.tile([P, N_ST * KB], mybir.dt.float32)
            for st in range(N_ST):
                t0 = st * G
                xt = xin_pool.tile([P, G, C], mybir.dt.float32)
                nc.sync.dma_start(out=xt[:], in_=xv[:, t0 : t0 + G, :])
                nc.vector.reduce_max(
                    out=m_all[:, t0 : t0 + G],
                    in_=xt[:],
                    axis=mybir.AxisListType.X,
                )
                # ACT path: exp+sum fused, one instruction per row
                esc = exp_pool.tile([P, 1, C], mybir.dt.float32, tag="esc")
                for j in range(KA):
                    nc.scalar.activation(
                        out=esc[:],
                        in_=xt[:, j : j + 1, :],
                        func=mybir.ActivationFunctionType.Exp,
                        accum_out=ua_all[:, st * KA + j : st * KA + j + 1],
                    )
                # DVE path: batched exp then grouped reduce_sum
                et = exp_pool.tile([P, KB, C], mybir.dt.float32, tag="et")
                nc.scalar.activation(
                    out=et[:],
                    in_=xt[:, KA:G, :],
                    func=mybir.ActivationFunctionType.Exp,
                )
                nc.vector.reduce_sum(
                    out=ub_all[:, st * KB : (st + 1) * KB],
                    in_=et[:],
                    axis=mybir.AxisListType.X,
                )
            nc.sync.dma_start(out=mv, in_=m_all[:])
            nc.sync.dma_start(out=uav, in_=ua_all[:])
            nc.sync.dma_start(out=ubv, in_=ub_all[:])
    nc.finalize()
    return nc


def run_device(y_pred: np.ndarray, **spmd_kwargs):
    """Run the bass kernel on 8 cores; returns (m, U) each [N] f32 plus results obj."""
    if "nc" not in _CACHE:
        _CACHE["nc"] = _build_bass()
    nc = _CACHE["nc"]
    in_maps = [{"x": y_pred[c * N_SHARD : (c + 1) * N_SHARD]} for c in range(N_CORES)]
    res = run_bass_kernel_spmd(nc, in_maps, core_ids=list(range(N_CORES)), **spmd_kwargs)
    m = np.concatenate([r["m_out"] for r in res.results])
    # reassemble U: per core, per partition, supertile st rows [0,KA) came from
    # the ACT path, rows [KA,G) from the DVE path
    u_parts = []
    for r in res.results:
        ua = r["ua_out"].reshape(P, N_ST, KA)
        ub = r["ub_out"].reshape(P, N_ST, KB)
        u = np.concatenate([ua, ub], axis=2)  # [P, N_ST, G]
        u_parts.append(u.reshape(P * T))
    u = np.concatenate(u_parts)
    return m, u, res


def finish_host(y_pred, y_true, m, u) -> np.ndarray:
    xl = y_pred[np.arange(N), np.asarray(y_true, dtype=np.int64)]
    conf = np.exp(m.astype(np.float64)) / u.astype(np.float64)
    acc = (xl == m).astype(np.float64)
    bin_idx = np.clip(np.ceil(conf * N_BINS).astype(np.int64) - 1, 0, N_BINS - 1)
    cnt = np.bincount(bin_idx, minlength=N_BINS).astype(np.float64)
    conf_sum = np.bincount(bin_idx, weights=conf, minlength=N_BINS)
    acc_sum = np.bincount(bin_idx, weights=acc, minlength=N_BINS)
    safe = np.where(cnt > 0, cnt, 1.0)
    per_bin = np.where(cnt > 0, np.abs(conf_sum / safe - acc_sum / safe) * (cnt / N), 0.0)
    return np.array([per_bin.sum()], dtype=np.float32)


def kernel(y_pred: np.ndarray, y_true: np.ndarray) -> np.ndarray:
    y_pred = np.ascontiguousarray(np.asarray(y_pred, dtype=np.float32))
    m, u, _ = run_device(y_pred)
    return finish_host(y_pred, y_true, m, u)


# revision 7
# speedup vs baseline: 1.2433x; 1.2433x over previous
"""ECE loss (equal-width 15-bin) for [1048576, 128] logits on 8 TRN2 NeuronCores.

Strategy (data-parallel over rows, per the sharding hint):
  Device, per core (N/8 = 131072 rows):
    - stream [128 partitions, G rows, 128 classes] supertiles of y_pred
    - DVE:   grouped reduce_max over classes -> per-row max m (all rows)
    - row softmax denominators U = sum_c exp(x_c) (unshifted exp is safe:
      |x| <= ~6.5), split between two engines to balance their load:
        * rows [0, KA) of each supertile: one ACT activation per row with
          accum_out -> exp+sum fused on the Scalar engine
        * rows [KA, G): one batched ACT exp + one grouped DVE reduce_sum
    - outputs m, u_a, u_b -- a 512MB -> 1.5MB reduction
  Host:
    conf = exp(m)/U  (== max softmax);  acc = (y_pred[r, y_true[r]] == m)
    (the row max is an exact element of the row, so float equality
    reproduces argmax == label up to exact-tie rows), then the 15-bin
    equal-width histogram and the final ECE reduction as in the reference.

Measured v1 (all reductions on DVE): 311us; DVE busy 282us was the
bottleneck while ACT sat at 119us and the DMA floor is ~195us. The KA
split moves ~half the row-sums to ACT's idle capacity.
"""

import numpy as np

import concourse.bacc as bacc
import concourse.tile as tile
from concourse import mybir
from concourse.bass_utils import run_bass_kernel_spmd

N_CORES = 8
N = 1048576
C = 128
N_SHARD = N // N_CORES  # 131072
P = 128                 # SBUF partitions
T = N_SHARD // P        # 1024 rows handled per partition
G = 32                  # rows per partition per supertile
N_ST = T // G           # supertiles per core
KA = 9                  # rows per supertile whose U is computed on ACT (accum)
KB = G - KA             # rows per supertile whose U is computed on DVE
N_BINS = 15

_CACHE: dict = {}


def _build_bass():
    nc = bacc.Bacc(None, target_bir_lowering=False)
    x = nc.dram_tensor("x", [N_SHARD, C], mybir.dt.float32, kind="ExternalInput")
    m_out = nc.dram_tensor("m_out", [N_SHARD], mybir.dt.float32, kind="ExternalOutput")
    ua_out = nc.dram_tensor("ua_out", [P * N_ST * KA], mybir.dt.float32, kind="ExternalOutput")
    ub_out = nc.dram_tensor("ub_out", [P * N_ST * KB], mybir.dt.float32, kind="ExternalOutput")

    # row r = p*T + t lives at [p, t]; per-partition runs in DRAM stay contiguous
    xv = x[:, :].rearrange("(p t) c -> p t c", p=P)
    mv = m_out[:].rearrange("(p t) -> p t", p=P)
    uav = ua_out[:].rearrange("(p t) -> p t", p=P)
    ubv = ub_out[:].rearrange("(p t) -> p t", p=P)

    with tile.TileContext(nc) as tc:
        with (
            tc.tile_pool(name="xin", bufs=5) as xin_pool,
            tc.tile_pool(name="exps", bufs=3) as exp_pool,
            tc.tile_pool(name="stats", bufs=1) as stats_pool,
        ):
            m_all = stats_pool.tile([P, T], mybir.dt.float32)
            ua_all = stats_pool.tile([P, N_ST * KA], mybir.dt.float32)
            ub_all = stats_pool.tile([P, N_ST * KB], mybir.dt.float32)
            for st in range(N_ST):
                t0 = st * G
                xt = xin_pool.tile([P, G, C], mybir.dt.float32)
                nc.sync.dma_start(out=xt[:], in_=xv[:, t0 : t0 + G, :])
                nc.vector.reduce_max(
                    out=m_all[:, t0 : t0 + G],
                    in_=xt[:],
                    axis=mybir.AxisListType.X,
                )
                # ACT path: exp+sum fused, one instruction per row
                esc = exp_pool.tile([P, 1, C], mybir.dt.float32, tag="esc")
                for j in range(KA):
                    nc.scalar.activation(
                        out=esc[:],
                        in_=xt[:, j : j + 1, :],
                        func=mybir.ActivationFunctionType.Exp,
                        accum_out=ua_all[:, st * KA + j : st * KA + j + 1],
                    )
                # DVE path: batched exp then grouped reduce_sum
                et = exp_pool.tile([P, KB, C], mybir.dt.float32, tag="et")
                nc.scalar.activation(
                    out=et[:],
                    in_=xt[:, KA:G, :],
                    func=mybir.ActivationFunctionType.Exp,
                )
                nc.vector.reduce_sum(
                    out=ub_all[:, st * KB : (st + 1) * KB],
                    in_=et[:],
                    axis=mybir.AxisListType.X,
                )
            nc.sync.dma_start(out=mv, in_=m_all[:])
            nc.sync.dma_start(out=uav, in_=ua_all[:])
            nc.sync.dma_start(out=ubv, in_=ub_all[:])
    nc.finalize()
    return nc


def run_device(y_pred: np.ndarray, **spmd_kwargs):
    """Run the bass kernel on 8 cores; returns (m, U) each [N] f32 plus results obj."""
    if "nc" not in _CACHE:
        _CACHE["nc"] = _build_bass()
    nc = _CACHE["nc"]
    in_maps = [{"x": y_pred[c * N_SHARD : (c + 1) * N_SHARD]} for c in range(N_CORES)]
    res = run_bass_kernel_spmd(nc, in_maps, core_ids=list(range(N_CORES)), **spmd_kwargs)
    m = np.concatenate([r["m_out"] for r in res.results])
    # reassemble U: per core, per partition, supertile st rows [0,KA) came from
    # the ACT path, rows [KA,G) from the DVE path
    u_parts = []
    for r in res.results:
        ua = r["ua_out"].reshape(P, N_ST, KA)
        ub = r["ub_out"].reshape(P, N_ST, KB)
        u = np.concatenate([ua, ub], axis=2)  # [P, N_ST, G]
        u_parts.append(u.reshape(P * T))
    u = np.concatenate(u_parts)
    return m, u, res


def finish_host(y_pred, y_true, m, u) -> np.ndarray:
    xl = y_pred[np.arange(N), np.asarray(y_true, dtype=np.int64)]
    conf = np.exp(m.astype(np.float64)) / u.astype(np.float64)
    acc = (xl == m).astype(np.float64)
    bin_idx = np.clip(np.ceil(conf * N_BINS).astype(np.int64) - 1, 0, N_BINS - 1)
    cnt = np.bincount(bin_idx, minlength=N_BINS).astype(np.float64)
    conf_sum = np.bincount(bin_idx, weights=conf, minlength=N_BINS)
    acc_sum = np.bincount(bin_idx, weights=acc, minlength=N_BINS)
    safe = np.where(cnt > 0, cnt, 1.0)
    per_bin = np.where(cnt > 0, np.abs(conf_sum / safe - acc_sum / safe) * (cnt / N), 0.0)
    return np.array([per_bin.sum()], dtype=np.float32)


def kernel(y_pred: np.ndarray, y_true: np.ndarray) -> np.ndarray:
    y_pred = np.ascontiguousarray(np.asarray(y_pred, dtype=np.float32))
    m, u, _ = run_device(y_pred)
    return finish_host(y_pred, y_true, m, u)
